# revision 4
# baseline (speedup 1.0000x reference)
"""Trainium2 Bass kernel for: y = k*tanh(x@w/d + b)[:,None] * w[None,:] + c*x.

Data-parallel over 8 NeuronCores: x is [16384, 4096] f32, sharded 2048
rows/core; w/c/k/b are tiny and folded host-side:
  wd = w/d            (dot-product weights; /d folded in)
  kw = k*w            (outer-product weights; k folded in)
  b  -> tanh bias (ACT immediate)
  c  -> if c != 1: feed x' = c*x and wd' = w/(d*c); identity otherwise.

Per-core device program (16 tiles of [128 rows, 4096 cols]):
  DMA in x_tile                                     (2 MB)
  DVE  tensor_tensor_reduce: t = x*wd, dot = sum(t) (1 pass)
  ACT  h = tanh(dot + b)                            ([128,1])
  DVE  scalar_tensor_tensor: y = (kw * h) + x       (1 pass)
  DMA out y_tile                                    (2 MB)

Memory-bound: 64 MB HBM traffic/core at ~358 GB/s -> ~180 us roofline.
"""

import os

import numpy as np

B = 16384
D = 4096
N_CORES = 8
P = 128
B_SHARD = B // N_CORES          # 2048 rows per core
N_TILES = B_SHARD // P          # 16 tiles per core

_CACHE = {}


def _build(add_x: bool, b_val: float):
    """Build + compile the per-core Bass program (SPMD, same graph on all cores)."""
    from contextlib import ExitStack

    import concourse.bass as bass  # noqa: F401  (registers engine classes)
    import concourse.tile as tile
    from concourse import bacc, mybir

    f32 = mybir.dt.float32
    nc = bacc.Bacc(
        "TRN2",
        debug=False,
        target_bir_lowering=False,
        num_devices=N_CORES,
    )

    x_ext = nc.dram_tensor("x", [B_SHARD, D], f32, kind="ExternalInput").ap()
    wd_ext = nc.dram_tensor("wd", [P, D], f32, kind="ExternalInput").ap()
    kw_ext = nc.dram_tensor("kw", [P, D], f32, kind="ExternalInput").ap()
    y_ext = nc.dram_tensor("y", [B_SHARD, D], f32, kind="ExternalOutput").ap()

    with tile.TileContext(nc) as tc, ExitStack() as ctx:
        consts = ctx.enter_context(tc.tile_pool(name="consts", bufs=1))
        xs = ctx.enter_context(tc.tile_pool(name="xs", bufs=3))
        ys = ctx.enter_context(tc.tile_pool(name="ys", bufs=3))
        ts = ctx.enter_context(tc.tile_pool(name="ts", bufs=2))
        ss = ctx.enter_context(tc.tile_pool(name="ss", bufs=4))

        wd_t = consts.tile([P, D], f32)
        nc.sync.dma_start(out=wd_t[:, :], in_=wd_ext[:, :])
        kw_t = consts.tile([P, D], f32)
        nc.sync.dma_start(out=kw_t[:, :], in_=kw_ext[:, :])
        bias_t = consts.tile([P, 1], f32)
        nc.gpsimd.memset(bias_t[:, :], float(b_val))

        def combine(x_t, h, r0):
            # y = kw * h + x, then DMA out
            y_t = ys.tile([P, D], f32)
            nc.vector.scalar_tensor_tensor(
                out=y_t[:, :],
                in0=kw_t[:, :],
                scalar=h[:, :],
                in1=x_t[:, :],
                op0=mybir.AluOpType.mult,
                op1=mybir.AluOpType.add if add_x else mybir.AluOpType.bypass,
            )
            nc.sync.dma_start(out=y_ext[r0 : r0 + P, :], in_=y_t[:, :])

        # Software-pipelined by one tile: the combine for tile i-1 is emitted
        # after tile i's dot pass, so the DVE never waits on ACT's tanh.
        prev = None
        for i in range(N_TILES):
            r0 = i * P
            x_t = xs.tile([P, D], f32)
            nc.sync.dma_start(out=x_t[:, :], in_=x_ext[r0 : r0 + P, :])

            # dot = sum(x * wd) per row (one fused DVE pass; `trash` unused)
            trash = ts.tile([P, D], f32)
            dot = ss.tile([P, 1], f32)
            nc.vector.scalar_tensor_tensor(
                out=trash[:, :],
                in0=x_t[:, :],
                scalar=1.0,
                in1=wd_t[:, :],
                op0=mybir.AluOpType.mult,
                op1=mybir.AluOpType.mult,
                accum_out=dot[:, :],
            )

            h = ss.tile([P, 1], f32)
            nc.scalar.activation(
                h[:, :],
                dot[:, :],
                mybir.ActivationFunctionType.Tanh,
                bias=bias_t[:, :],
                scale=1.0,
            )

            if prev is not None:
                combine(*prev)
            prev = (x_t, h, r0)
        combine(*prev)

    nc.compile()
    return nc


def _get_nc(add_x: bool, b_val: float):
    key = (add_x, float(b_val))
    if key not in _CACHE:
        _CACHE[key] = _build(add_x, b_val)
    return _CACHE[key]


# Results of the last traced run (set when BASS_KERNEL_TRACE=1): the
# BassKernelResults with exec_time_ns from the NTFF profile.
last_results = None


def kernel(x, w, c, k, b):
    from concourse.bass_utils import run_bass_kernel_spmd

    global last_results

    x = np.asarray(x, dtype=np.float32)
    w = np.asarray(w, dtype=np.float32).reshape(-1)
    c_val = float(np.asarray(c).reshape(-1)[0])
    k_val = float(np.asarray(k).reshape(-1)[0])
    b_val = float(np.asarray(b).reshape(-1)[0])
    assert x.shape == (B, D) and w.shape == (D,)

    add_x = c_val != 0.0
    if c_val not in (0.0, 1.0):
        x = c_val * x
        wd = w / (D * c_val)
    else:
        wd = w / D
    kw = k_val * w

    wd_b = np.ascontiguousarray(np.broadcast_to(wd[None, :], (P, D)), dtype=np.float32)
    kw_b = np.ascontiguousarray(np.broadcast_to(kw[None, :], (P, D)), dtype=np.float32)

    nc = _get_nc(add_x, b_val)

    in_maps = [
        {
            "x": np.ascontiguousarray(x[i * B_SHARD : (i + 1) * B_SHARD]),
            "wd": wd_b,
            "kw": kw_b,
        }
        for i in range(N_CORES)
    ]

    trace = os.environ.get("BASS_KERNEL_TRACE", "0") == "1"
    res = run_bass_kernel_spmd(
        nc, in_maps, core_ids=list(range(N_CORES)), trace=trace
    )
    last_results = res
    return np.concatenate([res.results[i]["y"] for i in range(N_CORES)], axis=0)


# revision 5
# speedup vs baseline: 1.9368x; 1.9368x over previous
"""Trainium2 Bass kernel for: y = k*tanh(x@w/d + b)[:,None] * w[None,:] + c*x.

Data-parallel over 8 NeuronCores: x is [16384, 4096] f32, sharded 2048
rows/core; w/c/k/b are tiny and folded host-side:
  wd = w/d            (dot-product weights; /d folded in)
  kw = k*w            (outer-product weights; k folded in)
  b  -> tanh bias
  c  -> if c != 1: feed x' = c*x and wd' = w/(d*c); identity otherwise.

Per-core device program (16 tiles of [128 rows, 4096 cols]):
  DMA in x_tile
  dot  = sum(x * wd) per row     (DVE mult [+ ACT accumulate-copy])
  h    = tanh(dot + b)           (ACT)
  y    = kw * h + x              (DVE tensor_scalar + add)
  DMA out y_tile

Memory-bound. Compute dtype is configurable: bf16 halves DMA traffic and
doubles DVE throughput at ~0.3% output error (well under tolerance).
"""

import os

import numpy as np

B = 16384
D = 4096
N_CORES = 8
P = 128
B_SHARD = B // N_CORES          # 2048 rows per core
N_TILES = B_SHARD // P          # 16 tiles per core

# variant knobs (resolved in _config)
DTYPE = os.environ.get("NK_DTYPE", "bf16")        # "f32" | "bf16"
DOT = os.environ.get("NK_DOT", "tt_act")          # "stt" | "tt_act"
COMB = os.environ.get("NK_COMB", "split")         # "stt" | "split" | "split_gp"
GP_COLS = int(os.environ.get("NK_GP_COLS", "0"))  # cols of final add on gpsimd
XBUFS = int(os.environ.get("NK_XBUFS", "4"))
YBUFS = int(os.environ.get("NK_YBUFS", "3"))

_CACHE = {}


def _build(add_x, b_val, dtype=DTYPE, dot=DOT, comb=COMB, gp_cols=GP_COLS,
           xbufs=XBUFS, ybufs=YBUFS, n_tiles=N_TILES):
    """Build + compile the per-core Bass program (SPMD, same graph on all cores)."""
    from contextlib import ExitStack

    import concourse.bass as bass  # noqa: F401
    import concourse.tile as tile
    from concourse import bacc, mybir

    f32 = mybir.dt.float32
    dt = mybir.dt.bfloat16 if dtype == "bf16" else f32
    rows = n_tiles * P

    nc = bacc.Bacc(
        "TRN2",
        debug=False,
        target_bir_lowering=False,
        num_devices=N_CORES,
    )

    x_ext = nc.dram_tensor("x", [rows, D], dt, kind="ExternalInput").ap()
    wd_ext = nc.dram_tensor("wd", [P, D], dt, kind="ExternalInput").ap()
    kw_ext = nc.dram_tensor("kw", [P, D], dt, kind="ExternalInput").ap()
    y_ext = nc.dram_tensor("y", [rows, D], dt, kind="ExternalOutput").ap()

    mult = mybir.AluOpType.mult
    add = mybir.AluOpType.add

    with tile.TileContext(nc) as tc, ExitStack() as ctx:
        consts = ctx.enter_context(tc.tile_pool(name="consts", bufs=1))
        xs = ctx.enter_context(tc.tile_pool(name="xs", bufs=xbufs))
        ys = ctx.enter_context(tc.tile_pool(name="ys", bufs=ybufs))
        ts = ctx.enter_context(tc.tile_pool(name="ts", bufs=2))
        ss = ctx.enter_context(tc.tile_pool(name="ss", bufs=4))

        wd_t = consts.tile([P, D], dt)
        nc.sync.dma_start(out=wd_t[:, :], in_=wd_ext[:, :])
        kw_t = consts.tile([P, D], dt)
        nc.sync.dma_start(out=kw_t[:, :], in_=kw_ext[:, :])
        bias_t = consts.tile([P, 1], f32)
        nc.gpsimd.memset(bias_t[:, :], float(b_val))

        def dot_pass(x_t):
            dotv = ss.tile([P, 1], f32)
            if dot == "stt":
                trash = ts.tile([P, D], dt)
                nc.vector.scalar_tensor_tensor(
                    out=trash[:, :], in0=x_t[:, :], scalar=1.0, in1=wd_t[:, :],
                    op0=mult, op1=mult, accum_out=dotv[:, :],
                )
            else:  # tt_act: DVE multiply, ACT accumulate-copy
                t1 = ts.tile([P, D], dt)
                nc.vector.tensor_mul(t1[:, :], x_t[:, :], wd_t[:, :])
                t2 = ts.tile([P, D], dt, tag="t2")
                nc.scalar.activation(
                    t2[:, :], t1[:, :], mybir.ActivationFunctionType.Copy,
                    accum_out=dotv[:, :],
                )
            h = ss.tile([P, 1], f32)
            nc.scalar.activation(
                h[:, :], dotv[:, :], mybir.ActivationFunctionType.Tanh,
                bias=bias_t[:, :], scale=1.0,
            )
            return h

        def combine(x_t, h, r0):
            y_t = ys.tile([P, D], dt)
            if comb == "stt":
                nc.vector.scalar_tensor_tensor(
                    out=y_t[:, :], in0=kw_t[:, :], scalar=h[:, :], in1=x_t[:, :],
                    op0=mult, op1=add if add_x else mybir.AluOpType.bypass,
                )
            else:
                y1 = ts.tile([P, D], dt, tag="y1")
                nc.vector.tensor_scalar(
                    out=y1[:, :], in0=kw_t[:, :], scalar1=h[:, :], scalar2=None,
                    op0=mult,
                )
                if not add_x:
                    y_t = y1
                elif comb == "split_gp" and gp_cols > 0:
                    cs = D - gp_cols
                    nc.vector.tensor_add(y_t[:, :cs], y1[:, :cs], x_t[:, :cs])
                    nc.gpsimd.tensor_add(y_t[:, cs:], y1[:, cs:], x_t[:, cs:])
                else:
                    nc.vector.tensor_add(y_t[:, :], y1[:, :], x_t[:, :])
            nc.sync.dma_start(out=y_ext[r0 : r0 + P, :], in_=y_t[:, :])

        # Software-pipelined by one tile: the combine for tile i-1 is emitted
        # after tile i's dot pass, so the DVE never waits on ACT's tanh.
        prev = None
        for i in range(n_tiles):
            r0 = i * P
            x_t = xs.tile([P, D], dt)
            nc.sync.dma_start(out=x_t[:, :], in_=x_ext[r0 : r0 + P, :])
            h = dot_pass(x_t)
            if prev is not None:
                combine(*prev)
            prev = (x_t, h, r0)
        combine(*prev)

    nc.compile()
    return nc


def _get_nc(add_x, b_val):
    key = (add_x, float(b_val))
    if key not in _CACHE:
        _CACHE[key] = _build(add_x, b_val)
    return _CACHE[key]


last_results = None


def kernel(x, w, c, k, b):
    import ml_dtypes
    from concourse.bass_utils import run_bass_kernel_spmd

    global last_results

    x = np.asarray(x, dtype=np.float32)
    w = np.asarray(w, dtype=np.float32).reshape(-1)
    c_val = float(np.asarray(c).reshape(-1)[0])
    k_val = float(np.asarray(k).reshape(-1)[0])
    b_val = float(np.asarray(b).reshape(-1)[0])
    assert x.shape == (B, D) and w.shape == (D,)

    add_x = c_val != 0.0
    if c_val not in (0.0, 1.0):
        x = c_val * x
        wd = w / (D * c_val)
    else:
        wd = w / D
    kw = k_val * w

    np_dt = ml_dtypes.bfloat16 if DTYPE == "bf16" else np.float32
    x_dev = x.astype(np_dt)
    wd_b = np.ascontiguousarray(
        np.broadcast_to(wd.astype(np_dt)[None, :], (P, D)))
    kw_b = np.ascontiguousarray(
        np.broadcast_to(kw.astype(np_dt)[None, :], (P, D)))

    nc = _get_nc(add_x, b_val)

    in_maps = [
        {
            "x": np.ascontiguousarray(x_dev[i * B_SHARD : (i + 1) * B_SHARD]),
            "wd": wd_b,
            "kw": kw_b,
        }
        for i in range(N_CORES)
    ]

    trace = os.environ.get("BASS_KERNEL_TRACE", "0") == "1"
    res = run_bass_kernel_spmd(
        nc, in_maps, core_ids=list(range(N_CORES)), trace=trace
    )
    last_results = res
    y = np.concatenate([res.results[i]["y"] for i in range(N_CORES)], axis=0)
    return y.astype(np.float32)
